# revision 53
# baseline (speedup 1.0000x reference)
"""Trainium2 Bass kernel for nn_Attention (B=4, S=2048, D=1024, H=16) on 8 NeuronCores.

Sharding: data-parallel over (batch, sequence-half) -> 8 shards, one per core.
Each core computes attention for 1024 query tokens of one batch element.

v2: fp8 rework of the bf16 baseline.
 - All projections (Q/K/V/O) run as fp8e4m3 DoubleRow matmuls: contraction of
   2x128 partitions per instruction at the same issue rate as bf16, i.e. 2x
   the throughput.
 - QK^T stays bf16 but alternates the 64-partition stationary base between
   head 0 (partitions 0:64) and head 1 (64:128) of each pair, so the PE
   loads one head's keys while streaming the other's queries (~2x issue rate).
 - Attention weights A = exp(s/8 - 4?) are stored in fp8e5m2 (wide dynamic
   range; raw scores reach 73 > e4m3 budget). The exp stream - the biggest
   elementwise cost - is split between ScalarE (exact exp, fp8 output) and
   DVE (Schraudolph: affine in log2 domain + saturating round to uint8 IS the
   fp8e5m2 bit pattern). A@V contracts A (e5m2) against V (e4m3) in DoubleRow
   mode with a ones-column folding the softmax denominator into the matmul.
 - Denominators collect into [8,S] tiles; batched reciprocal + per-pair
   broadcast matmul (sel8) normalizes O^T, written fp8 for the O projection.
 - Residual + LayerNorm in fp32; the (x-mu)*rstd affine runs on ScalarE with
   per-partition scale/bias APs.
Zero biases / identity gamma,beta (checked on host) skip their instructions.
"""

import os
import sys

sys.path.insert(0, "/opt/trn_rl_repo")

import numpy as np

B, S, D, H = 4, 2048, 1024, 16
HD = D // H  # 64
SQ = S // 2  # queries per core
NCORES = 8
EPS = 1e-12

SHIFT = 2.5
K_OCT = 2  # global 2^-K scale on A: keeps e4m3 under NaN/448 for scores ~75
LOG2E = 1.4426950408889634
LN2 = 0.6931471805599453
NSHIFT = -(SHIFT + K_OCT * LN2)
A_DVE = LOG2E * 0.125 * 8               # schraudolph slope per raw score (e4m3)
B_DVE = 56.0 - 0.45 - 8 * SHIFT * LOG2E - 8 * K_OCT  # fitted offset (c=-0.45)

_CACHE = {}


def _install_ntff_hook():
    """Register the axon NTFF profile hook that bass_utils looks up via
    antenv.axon_hooks (absent from the image's antenv stub)."""
    import contextlib
    import ctypes
    import types

    so_path = "/opt/axon/libaxon_pjrt.so"
    if "antenv.axon_hooks" in sys.modules:
        return
    try:
        lib = ctypes.CDLL(so_path)
    except OSError:
        return
    if not hasattr(lib, "axon_start_nrt_profile"):
        return
    lib.axon_start_nrt_profile.argtypes = [ctypes.POINTER(ctypes.c_int64), ctypes.c_size_t]
    lib.axon_start_nrt_profile.restype = ctypes.c_int64
    lib.axon_stop_nrt_profile.argtypes = [ctypes.c_char_p]
    lib.axon_stop_nrt_profile.restype = ctypes.c_int64

    @contextlib.contextmanager
    def _hook(output_dir, device_ids):
        import jax

        jax.devices()
        if device_ids:
            ids = (ctypes.c_int64 * len(device_ids))(*device_ids)
            rc = lib.axon_start_nrt_profile(ids, len(device_ids))
        else:
            rc = lib.axon_start_nrt_profile(None, 0)
        if rc != 0:
            raise RuntimeError(f"axon_start_nrt_profile rc={rc}")
        try:
            yield
        finally:
            n = lib.axon_stop_nrt_profile(str(output_dir).encode())
            if n < 0:
                raise RuntimeError(f"axon_stop_nrt_profile rc={n}")

    m = types.ModuleType("antenv.axon_hooks")
    m.get_axon_ntff_profile_hook = lambda: _hook
    m.set_axon_ntff_profile_hook = lambda h: None
    sys.modules["antenv.axon_hooks"] = m


def _build(flags):
    use_bq, use_bk, use_bv, use_bo, use_gamma, use_beta = flags

    import concourse.bass as bass
    import concourse.tile as tile
    from concourse import bacc, mybir

    f32 = mybir.dt.float32
    bf16 = mybir.dt.bfloat16
    fp8 = mybir.dt.float8e4
    fp8e5 = mybir.dt.float8e5
    f32r = mybir.dt.float32r
    u8 = mybir.dt.uint8
    ADD = mybir.AluOpType.add
    MULT = mybir.AluOpType.mult
    SUB = mybir.AluOpType.subtract
    Exp = mybir.ActivationFunctionType.Exp
    Sqrt = mybir.ActivationFunctionType.Sqrt
    Copy = mybir.ActivationFunctionType.Copy
    Ident = mybir.ActivationFunctionType.Identity
    DR = mybir.MatmulPerfMode.DoubleRow

    nc = bacc.Bacc("TRN2")

    xt_d = nc.dram_tensor("xt", [D, S], fp8, kind="ExternalInput")
    xq_d = nc.dram_tensor("xq", [SQ, D], bf16, kind="ExternalInput")
    wq_d = nc.dram_tensor("wqt", [D, D], fp8, kind="ExternalInput")
    wk_d = nc.dram_tensor("wkt", [D, D], fp8, kind="ExternalInput")
    wv_d = nc.dram_tensor("wvt", [D, D], fp8, kind="ExternalInput")
    wo_d = nc.dram_tensor("wot", [D, D], fp8, kind="ExternalInput")
    bq_d = nc.dram_tensor("bqt", [128, 8], f32, kind="ExternalInput")
    bk_d = nc.dram_tensor("bkt", [128, 8], f32, kind="ExternalInput")
    bv_d = nc.dram_tensor("bv", [D], f32, kind="ExternalInput")
    bo_d = nc.dram_tensor("bo", [D], f32, kind="ExternalInput")
    gamma_d = nc.dram_tensor("gamma", [D], f32, kind="ExternalInput")
    beta_d = nc.dram_tensor("beta", [D], f32, kind="ExternalInput")
    sel8_d = nc.dram_tensor("sel8", [8, 4, 128], bf16, kind="ExternalInput")
    ident_d = nc.dram_tensor("ident", [128, 128], bf16, kind="ExternalInput")
    out_d = nc.dram_tensor("out", [SQ, D], f32, kind="ExternalOutput")

    def bcast_ap(handle):
        ap = handle[:]
        return bass.AP(tensor=ap.tensor, offset=ap.offset, ap=[[0, 128], ap.ap[0]])

    # which (hh, kc) score tiles go to ScalarE (exact exp) vs DVE (schraudolph)
    scalar_set = {i for i in range(32) if (i * 17) % 32 < 16}

    with tile.TileContext(nc) as tc:
        with (
            tc.tile_pool(name="const", bufs=1) as constp,
            tc.tile_pool(name="v", bufs=1) as vp,
            tc.tile_pool(name="ot", bufs=1) as otp,
            tc.tile_pool(name="xt", bufs=1) as xtp,
            tc.tile_pool(name="wo", bufs=1) as wop,
            tc.tile_pool(name="rc", bufs=1) as rcp,
        ):
            # --- constants ---
            bq_c = constp.tile([128, 8], f32, tag="bq")
            bk_c = constp.tile([128, 8], f32, tag="bk")
            eps_c = constp.tile([128, 1], f32, tag="eps")
            nshift_c = constp.tile([128, 1], f32, tag="nshift")
            scratch_c = constp.tile([128, 1], f32, tag="scratch")
            sel8_c = constp.tile([8, 4, 128], bf16, tag="sel8")
            ident = constp.tile([128, 128], bf16, tag="ident")
            bv_c = constp.tile([128, D], f32, tag="bv") if use_bv else None
            gamma_c = constp.tile([128, D], f32, tag="gamma") if use_gamma else None
            beta_c = constp.tile([128, D], f32, tag="beta") if use_beta else None
            bo_c = constp.tile([128, D], f32, tag="bo") if use_bo else None
            if use_bq:
                nc.sync.dma_start(out=bq_c[:], in_=bq_d[:])
            if use_bk:
                nc.sync.dma_start(out=bk_c[:], in_=bk_d[:])
            if use_bv:
                nc.gpsimd.dma_start(out=bv_c[:], in_=bcast_ap(bv_d))
            if use_bo:
                nc.gpsimd.dma_start(out=bo_c[:], in_=bcast_ap(bo_d))
            if use_gamma:
                nc.gpsimd.dma_start(out=gamma_c[:], in_=bcast_ap(gamma_d))
            if use_beta:
                nc.gpsimd.dma_start(out=beta_c[:], in_=bcast_ap(beta_d))
            nc.sync.dma_start(out=sel8_c[:], in_=sel8_d[:])
            nc.gpsimd.dma_start(out=ident[:], in_=ident_d[:])
            nc.vector.memset(eps_c[:], EPS)
            nc.vector.memset(nshift_c[:], NSHIFT)

            # --- persistent activations ---
            v8 = vp.tile([128, 16, H, HD + 1], fp8, tag="v")   # V + ones col (den)
            otb = otp.tile([128, 8, SQ], bf16, tag="otb")      # O^T unnormalized
            ot8 = otp.tile([128, 8, SQ], fp8, tag="ot8")       # O^T normalized
            den_a = otp.tile([8, SQ], f32, tag="den_a")        # heads 0-7
            den_b = otp.tile([4, SQ], f32, tag="den_b")        # heads 8-11
            den_c = otp.tile([4, SQ], f32, tag="den_c")        # heads 12-15
            xt = xtp.tile([128, 8, S], fp8, tag="xt")
            wo_r = wop.tile([128, 8, D], fp8, tag="wor")
            wv_r = wop.tile([128, 8, D], fp8, tag="wvr")
            xq_s = wop.tile([128, 8, D], bf16, tag="xq")

            nc.vector.memset(v8[:, :, :, HD : HD + 1], 1.0)
            # Single-post bulk loads: DMA posts cost ~700ns of engine time
            # each, so one 3D-AP transfer per tensor. pair-0 Q/K weights go
            # first on their queues (they gate the first matmuls).
            dq = [nc.sync, nc.scalar, nc.gpsimd]
            wq_0 = wop.tile([128, 8, 128], fp8, tag="wq0")
            wk_0 = wop.tile([128, 8, 128], fp8, tag="wk0")
            nc.sync.dma_start(
                out=wq_0[:], in_=wq_d[:, 0:128].rearrange("(k p) c -> p k c", p=128)
            )
            nc.scalar.dma_start(
                out=wk_0[:], in_=wk_d[:, 0:128].rearrange("(k p) c -> p k c", p=128)
            )
            # xt gates the first matmuls: keep sync/scalar queues clear for it;
            # wv (needed ~kc0 of pair 0) and wo (phase 3) ride on gpsimd.
            # xt rows spread over all three queues; wv/wo trail on gpsimd
            # (v_chain starts at kc 2, wo only needed in phase 3).
            for r in range(8):
                dq[r % 3].dma_start(
                    out=xt[:, r, :], in_=xt_d[r * 128 : (r + 1) * 128, :]
                )
            for k in range(4):
                nc.gpsimd.dma_start(
                    out=wv_r[:, 2 * k : 2 * k + 2, :],
                    in_=wv_d[2 * k * 128 : (2 * k + 2) * 128, :].rearrange(
                        "(k p) c -> p k c", p=128
                    ),
                )
            for k in range(4):
                nc.gpsimd.dma_start(
                    out=wo_r[:, 2 * k : 2 * k + 2, :],
                    in_=wo_d[2 * k * 128 : (2 * k + 2) * 128, :].rearrange(
                        "(k p) c -> p k c", p=128
                    ),
                )

            with (
                tc.tile_pool(name="qkw", bufs=2) as qkwp,
                tc.tile_pool(name="qts", bufs=2) as qtsp,
                tc.tile_pool(name="kts", bufs=2) as ktsp,
                tc.tile_pool(name="st", bufs=16) as stp,
                tc.tile_pool(name="stage", bufs=4) as stagep,
                tc.tile_pool(name="stgd", bufs=2) as stgdp,
                tc.tile_pool(name="ps1", bufs=1, space="PSUM") as ps1,
                tc.tile_pool(name="sp", bufs=3, space="PSUM") as spp,
                tc.tile_pool(name="av", bufs=1, space="PSUM") as avp,
            ):
                # ---------- piecewise emission helpers ----------
                def v_chain(tc_i, dg):
                    pool, tg = (ps1, "ps") if (2 * tc_i + dg) % 2 == 0 else (avp, "av")
                    psv = pool.tile([128, 512], f32, tag=tg, name="psv")
                    for k in range(4):
                        nc.tensor.matmul(
                            out=psv[:],
                            lhsT=xt[:, 2 * k : 2 * k + 2, tc_i * 128 : (tc_i + 1) * 128],
                            rhs=wv_r[:, 2 * k : 2 * k + 2, dg * 512 : (dg + 1) * 512],
                            start=(k == 0),
                            stop=(k == 3),
                            perf_mode=DR,
                        )
                    dst = v8[:, tc_i, dg * 8 : (dg + 1) * 8, 0:HD]
                    if use_bv:
                        nc.vector.tensor_tensor(
                            out=dst,
                            in0=psv[:].rearrange("p (h d) -> p h d", d=HD),
                            in1=bv_c[:, dg * 512 : (dg + 1) * 512].rearrange(
                                "p (h d) -> p h d", d=HD
                            ),
                            op=ADD,
                        )
                    else:
                        nc.vector.tensor_copy(
                            out=dst, in_=psv[:].rearrange("p (h d) -> p h d", d=HD)
                        )

                pair_qt = {}

                def proj_piece(m, j):
                    """j=0: wq DMA + Q chain tg0; j=1: Q tg1; j=2: wk DMA + K tg0;
                    j=3..5: K tg1..3."""
                    st = pair_qt.setdefault(m, {})
                    if j == 0:
                        if m == 0:
                            st["wq"] = wq_0
                        else:
                            wq_m = qkwp.tile([128, 8, 128], fp8, tag="qkw", name="wq_m")
                            nc.sync.dma_start(
                                out=wq_m[:],
                                in_=wq_d[:, m * 128 : (m + 1) * 128].rearrange(
                                    "(k p) c -> p k c", p=128
                                ),
                            )
                            st["wq"] = wq_m
                        st["qt"] = qtsp.tile([128, SQ], bf16, tag="qts", name="qt_m")
                    if j == 2:
                        if m == 0:
                            st["wk"] = wk_0
                        else:
                            wk_m = qkwp.tile([128, 8, 128], fp8, tag="qkw", name="wk_m")
                            nc.sync.dma_start(
                                out=wk_m[:],
                                in_=wk_d[:, m * 128 : (m + 1) * 128].rearrange(
                                    "(k p) c -> p k c", p=128
                                ),
                            )
                            st["wk"] = wk_m
                        st["kt"] = ktsp.tile([128, S], bf16, tag="kts", name="kt_m")
                    if j < 2:
                        w, dstt, tg, bias_c, use_b = st["wq"], st["qt"], j, bq_c, use_bq
                    else:
                        w, dstt, tg, bias_c, use_b = st["wk"], st["kt"], j - 2, bk_c, use_bk
                    ps = ps1.tile([128, 512], f32, tag="ps", name="psqk")
                    for k in range(4):
                        nc.tensor.matmul(
                            out=ps[:],
                            lhsT=w[:, 2 * k : 2 * k + 2, :],
                            rhs=xt[:, 2 * k : 2 * k + 2, tg * 512 : (tg + 1) * 512],
                            start=(k == 0),
                            stop=(k == 3),
                            perf_mode=DR,
                        )
                    if use_b:
                        nc.scalar.activation(
                            out=dstt[:, tg * 512 : (tg + 1) * 512],
                            in_=ps[:],
                            func=Ident,
                            bias=bias_c[:, m : m + 1],
                        )
                    else:
                        nc.scalar.copy(
                            out=dstt[:, tg * 512 : (tg + 1) * 512], in_=ps[:]
                        )

                def qk_exp_kc(m, kc, qt_m, kt_m, st_pair):
                    sps = [
                        spp.tile([128, 1024], f32, tag="sp", name="sp") for _ in range(2)
                    ]
                    for hh in range(2):
                        p0 = hh * 64
                        for qh in range(2):
                            nc.tensor.matmul(
                                out=sps[hh][:, qh * 512 : (qh + 1) * 512],
                                lhsT=kt_m[p0 : p0 + 64, kc * 128 : (kc + 1) * 128],
                                rhs=qt_m[p0 : p0 + 64, qh * 512 : (qh + 1) * 512],
                                start=True,
                                stop=True,
                            )
                        dst = st_pair[hh][kc // 4][:, kc % 4, :]
                        if (2 * kc + hh) in scalar_set:
                            nc.scalar.activation(
                                out=dst,
                                in_=sps[hh][:],
                                func=Exp,
                                scale=0.125,
                                bias=nshift_c[:, 0:1],
                            )
                        else:
                            nc.vector.tensor_scalar(
                                out=dst.bitcast(u8),
                                in0=sps[hh][:],
                                scalar1=float(A_DVE),
                                scalar2=float(B_DVE),
                                op0=MULT,
                                op1=ADD,
                            )

                av_stg = {}
                av_live = {}

                def av_half(m, half_i, st_pair, tail=False):
                    """half_i = 0..7: piece (hh,qh) split into two 4-accum halves."""
                    piece, half = half_i // 2, half_i % 2
                    den_t = den_a if m < 4 else (den_b if m < 6 else den_c)
                    den_r = 2 * (m % 4) if m < 4 else (2 * (m - 4) if m < 6 else 2 * (m - 6))
                    hh, qh = piece // 2, piece % 2
                    h = 2 * m + hh
                    st_tiles = st_pair[hh]
                    if qh == 0 and half == 0:
                        av_stg[(m, hh)] = stagep.tile(
                            [65, 2, 512], bf16, tag="stg", name="stg"
                        )
                        av_stg[(m, hh, "d")] = stgdp.tile(
                            [65, 2, 512], f32, tag="stgd", name="stgd"
                        )
                    stg = av_stg[(m, hh)]
                    stgd = av_stg[(m, hh, "d")]
                    if half == 0:
                        if tail:
                            # QK is done; reuse a freed score bank so tail
                            # pieces pipeline instead of serializing on avp.
                            av_live[m] = spp.tile([128, 1024], f32, tag="sp", name="sp")
                        else:
                            av_live[m] = avp.tile([128, 512], f32, tag="av", name="av")
                    av = av_live[m]
                    for c in range(4 * half, 4 * half + 4):
                        u, jj = c // 2, c % 2
                        nc.tensor.matmul(
                            out=av[0:65, 0:512],
                            lhsT=v8[:, 4 * u + 2 * jj : 4 * u + 2 * jj + 2, h, :],
                            rhs=st_tiles[u][:, 2 * jj : 2 * jj + 2, qh * 512 : (qh + 1) * 512],
                            start=(c == 0),
                            stop=(c == 7),
                            perf_mode=DR,
                            skip_group_check=True,
                        )
                    if half == 0:
                        return
                    if hh == 0:
                        nc.vector.tensor_copy(
                            out=otb[0:64, m, qh * 512 : (qh + 1) * 512],
                            in_=av[0:64, 0:512],
                        )
                    else:
                        nc.scalar.copy(out=stg[0:64, qh, :], in_=av[0:64, 0:512])
                    nc.scalar.copy(out=stgd[64:65, qh, :], in_=av[64:65, 0:512])
                    if qh == 1:
                        nc.sync.dma_start(
                            out=den_t[den_r + hh : den_r + hh + 1, :],
                            in_=stgd[64:65, :, :],
                        )
                        if hh == 1:
                            nc.sync.dma_start(
                                out=otb[64:128, m, :], in_=stg[0:64, :, :]
                            )

                NORM_PAIRS = ((0, 1, 2, 3), (4, 5), (6, 7))
                norm_rc = {}

                def norm_prep(b_i, r0=0, r1=None):
                    # DVE-side reciprocal for a den batch; emitted at pair end
                    den_t = (den_a, den_b, den_c)[b_i]
                    if r1 is None:
                        r1 = (8, 4, 4)[b_i]
                    rc_f = rcp.tile([8, SQ], f32, tag="rcf", name="rc_f")
                    rc_b = rcp.tile([8, SQ], bf16, tag="rcb", name="rc_b")
                    nc.vector.reciprocal_approx_fast(
                        out=rc_f[r0:r1, :], in_=den_t[r0:r1, :]
                    )
                    # SBUF->SBUF cast on the otherwise-idle Pool engine
                    nc.gpsimd.tensor_copy(out=rc_b[r0:r1, :], in_=rc_f[r0:r1, :])
                    norm_rc[b_i] = rc_b

                def norm_bc(b_i, i, pool=None, ptag="ps"):
                    # one broadcast matmul + normalize; spread into later kc slots
                    pairs = NORM_PAIRS[b_i]
                    m, qh = pairs[i // 2], i % 2
                    mm = m % 4 if b_i == 0 else m - (4 if b_i == 1 else 6)
                    rc_b = norm_rc[b_i]
                    bc = (pool or ps1).tile([128, 512], f32, tag=ptag, name="bc")
                    nc.tensor.matmul(
                        out=bc[:],
                        lhsT=sel8_c[:, mm, :],
                        rhs=rc_b[:, qh * 512 : (qh + 1) * 512],
                        start=True,
                        stop=True,
                    )
                    nc.vector.tensor_tensor(
                        out=ot8[:, m, qh * 512 : (qh + 1) * 512],
                        in0=otb[:, m, qh * 512 : (qh + 1) * 512],
                        in1=bc[:],
                        op=MULT,
                    )

                # ---------- interleaved pipeline ----------
                pair_st = {}
                vq = [(tc_i, dg) for tc_i in range(16) for dg in range(2)]
                # only Q (j=0,1) and K tg0 (j=2) up front; K tg1..3 fold into
                # kc 1/3/5 of pair 0 so QK can start on partial kt.
                for jj in range(3):
                    proj_piece(0, jj)
                for m in range(8):
                    qt_m = pair_qt[m]["qt"]
                    kt_m = pair_qt[m]["kt"]
                    st_pair = [
                        [stp.tile([128, 4, SQ], fp8, tag="st", name="st") for _ in range(4)]
                        for _ in range(2)
                    ]
                    pair_st[m] = st_pair
                    for kc in range(16):
                        qk_exp_kc(m, kc, qt_m, kt_m, st_pair)
                        if m == 0:
                            if kc in (1, 3, 5):
                                proj_piece(0, 3 + kc // 2)
                            # fold the V projection into pair 0's loop
                            # (start at kc 2 so wv's DMA has landed)
                            nv = 0 if kc < 2 else (3 if kc < 6 else 2)
                            for _ in range(nv):
                                if vq:
                                    v_chain(*vq.pop(0))
                        if m >= 1 and kc % 2 == 1:
                            av_half(m - 1, kc // 2, pair_st[m - 1])
                        if m == 5 and kc % 2 == 1:
                            norm_bc(0, kc // 2)
                        if m == 7 and kc in (3, 5, 7, 9):
                            norm_bc(1, (kc - 3) // 2)
                        if m < 7 and kc % 2 == 0 and kc < 12:
                            proj_piece(m + 1, kc // 2)
                        if m == 5 and kc == 0:
                            # prefetch X rows for the phase-3 residual
                            nc.gpsimd.dma_start(
                                out=xq_s[:],
                                in_=xq_d[:].rearrange("(t p) c -> p t c", p=128),
                            )
                    if m >= 2:
                        del pair_st[m - 2]
                    if m == 4:
                        norm_prep(0)
                    if m == 6:
                        norm_prep(1)
                norm_prep(2, 0, 2)  # pair 6 (den ready); pair 7 comes in phase 3
                for hi in range(8):
                    av_half(7, hi, pair_st[7], tail=True)
                    if hi in (3, 5):
                        norm_bc(2, (hi - 3) // 2)
                # pull the sqrt activation table in before phase 3 needs it
                nc.scalar.activation(
                    out=scratch_c[:], in_=eps_c[:], func=Sqrt, bias=eps_c[:, 0:1]
                )

            # ========== phase 3: O projection + residual + LN ==========
            with (
                tc.tile_pool(name="yo", bufs=4) as yop,
                tc.tile_pool(name="stats", bufs=4) as statp,
                tc.tile_pool(name="ps3", bufs=4, space="PSUM") as ps3,
            ):
                if use_bo:
                    for t in range(8):
                        nc.gpsimd.tensor_tensor(
                            out=xq_s[:, t, :], in0=xq_s[:, t, :], in1=bo_c[:], op=ADD
                        )
                def oproj_mm(ps, t, g, k):
                    nc.tensor.matmul(
                        out=ps[:, g * 512 : (g + 1) * 512],
                        lhsT=ot8[:, 2 * k : 2 * k + 2, t * 128 : (t + 1) * 128],
                        rhs=wo_r[:, 2 * k : 2 * k + 2, g * 512 : (g + 1) * 512],
                        start=(k == 0),
                        stop=False,
                        perf_mode=DR,
                        skip_group_check=True,
                    )

                # allocate bc7's PSUM slot ahead of t0 so it recycles first
                # and the t-loop keeps full 4-deep ps3 rotation
                bc7 = ps3.tile([128, D], f32, tag="ps3", name="bc7")
                for t in range(8):
                    ps = ps3.tile([128, D], f32, tag="ps3", name="ps3")
                    stats = statp.tile([128, 2, 6], f32, tag="stats")
                    mv = statp.tile([128, 2], f32, tag="mv")
                    mr = statp.tile([128, 1], f32, tag="mr")
                    if t == 0:
                        # pairs 0-5 first: pair-7 normalization in flight
                        for g in range(2):
                            for k in range(3):
                                oproj_mm(ps, t, g, k)
                        # (redo rows 0:2 too: DVE APs must start at partition 0)
                        norm_prep(2, 0, 4)
                        for qh in range(2):
                            nc.tensor.matmul(
                                out=bc7[:, qh * 512 : (qh + 1) * 512],
                                lhsT=sel8_c[:, 1, :],
                                rhs=norm_rc[2][:, qh * 512 : (qh + 1) * 512],
                                start=True,
                                stop=True,
                            )
                            nc.vector.tensor_tensor(
                                out=ot8[:, 7, qh * 512 : (qh + 1) * 512],
                                in0=otb[:, 7, qh * 512 : (qh + 1) * 512],
                                in1=bc7[:, qh * 512 : (qh + 1) * 512],
                                op=MULT,
                            )
                        for g in range(2):
                            oproj_mm(ps, t, g, 3)
                            nc.tensor.matmul(
                                out=ps[:, g * 512 : (g + 1) * 512],
                                lhsT=ident[:],
                                rhs=xq_s[:, t, g * 512 : (g + 1) * 512],
                                start=False,
                                stop=True,
                                skip_group_check=True,
                            )
                            nc.vector.bn_stats(
                                out=stats[:, g, :],
                                in_=ps[:, g * 512 : (g + 1) * 512],
                            )
                    else:
                        # g-major: half-g stats overlap the other half's matmuls
                        for g in range(2):
                            for k in range(4):
                                oproj_mm(ps, t, g, k)
                            # residual: accumulate X via bf16 identity matmul
                            nc.tensor.matmul(
                                out=ps[:, g * 512 : (g + 1) * 512],
                                lhsT=ident[:],
                                rhs=xq_s[:, t, g * 512 : (g + 1) * 512],
                                start=False,
                                stop=True,
                                skip_group_check=True,
                            )
                            nc.vector.bn_stats(
                                out=stats[:, g, :],
                                in_=ps[:, g * 512 : (g + 1) * 512],
                            )
                    nc.vector.bn_aggr(out=mv[:], in_=stats[:])
                    nc.scalar.activation(
                        out=mv[:, 1:2], in_=mv[:, 1:2], func=Sqrt, bias=eps_c[:, 0:1]
                    )
                    nc.vector.reciprocal(out=mv[:, 1:2], in_=mv[:, 1:2])
                    nc.vector.tensor_scalar(
                        out=mr[:],
                        in0=mv[:, 0:1],
                        scalar1=mv[:, 1:2],
                        scalar2=-1.0,
                        op0=MULT,
                        op1=MULT,
                    )
                    yo = yop.tile([128, D], f32, tag="yo")
                    nc.scalar.activation(
                        out=yo[:], in_=ps[:], func=Ident,
                        scale=mv[:, 1:2], bias=mr[:, 0:1],
                    )
                    if use_gamma:
                        nc.vector.tensor_tensor(out=yo[:], in0=yo[:], in1=gamma_c[:], op=MULT)
                    if use_beta:
                        nc.gpsimd.tensor_tensor(out=yo[:], in0=yo[:], in1=beta_c[:], op=ADD)
                    dq[t % 3].dma_start(
                        out=out_d[t * 128 : (t + 1) * 128, :], in_=yo[:]
                    )

    nc.compile()
    return nc


def _get_nc(flags):
    key = ("nc", flags)
    if key not in _CACHE:
        _CACHE[key] = _build(flags)
    return _CACHE[key]


def kernel(X, Wq, bq, Wk, bk, Wv, bv, Wo, bo, gamma, beta):
    if os.environ.get("BASS_TRACE"):
        _install_ntff_hook()
    import ml_dtypes

    from concourse.bass_utils import run_bass_kernel_spmd

    f8 = ml_dtypes.float8_e4m3
    bfdt = ml_dtypes.bfloat16
    f32 = np.float32
    X = np.ascontiguousarray(np.asarray(X, dtype=f32))
    bq_ = np.asarray(bq, f32)
    bk_ = np.asarray(bk, f32)
    bv_ = np.ascontiguousarray(np.asarray(bv, f32))
    bo_ = np.ascontiguousarray(np.asarray(bo, f32))
    gamma_ = np.ascontiguousarray(np.asarray(gamma, f32))
    beta_ = np.ascontiguousarray(np.asarray(beta, f32))
    flags = (
        bool(np.any(bq_)), bool(np.any(bk_)), bool(np.any(bv_)), bool(np.any(bo_)),
        bool(np.any(gamma_ != 1.0)), bool(np.any(beta_)),
    )

    wqt = np.ascontiguousarray(np.asarray(Wq, f32).T.astype(f8))
    wkt = np.ascontiguousarray(np.asarray(Wk, f32).T.astype(f8))
    wvt = np.ascontiguousarray(np.asarray(Wv, f32).T.astype(f8))
    wot = np.ascontiguousarray(np.asarray(Wo, f32).T.astype(f8))
    bqt = np.ascontiguousarray(bq_.reshape(8, 128).T)
    bkt = np.ascontiguousarray(bk_.reshape(8, 128).T)
    sel8 = np.zeros((8, 4, 128), f32)
    for mm in range(4):
        for p in range(128):
            sel8[2 * mm + p // 64, mm, p] = 1.0
    sel8 = sel8.astype(bfdt)
    ident = np.eye(128, dtype=f32).astype(bfdt)

    in_maps = []
    for c in range(NCORES):
        b, half = c // 2, c % 2
        Xb = X[b]
        q_rows = Xb[half * SQ : (half + 1) * SQ]
        o_rows = Xb[(1 - half) * SQ : (2 - half) * SQ]
        # queries-first token order (key order is permutation-invariant)
        xt = np.ascontiguousarray(np.concatenate([q_rows, o_rows], axis=0).T.astype(f8))
        in_maps.append(
            {
                "xt": xt,
                "xq": np.ascontiguousarray(q_rows.astype(bfdt)),
                "wqt": wqt,
                "wkt": wkt,
                "wvt": wvt,
                "wot": wot,
                "bqt": bqt,
                "bkt": bkt,
                "bv": bv_,
                "bo": bo_,
                "gamma": gamma_,
                "beta": beta_,
                "sel8": sel8,
                "ident": ident,
            }
        )

    nc = _get_nc(flags)
    res = run_bass_kernel_spmd(nc, in_maps, core_ids=list(range(NCORES)))
    if res.exec_time_ns is not None:
        print(f"HW exec time: {res.exec_time_ns} ns")

    out = np.empty((B, S, D), np.float32)
    for c in range(NCORES):
        b, half = c // 2, c % 2
        out[b, half * SQ : (half + 1) * SQ] = res.results[c]["out"]
    return out



# revision 55
# speedup vs baseline: 1.0340x; 1.0340x over previous
"""Trainium2 Bass kernel for nn_Attention (B=4, S=2048, D=1024, H=16) on 8 NeuronCores.

Sharding: data-parallel over (batch, sequence-half) -> 8 shards, one per core.
Each core computes attention for 1024 query tokens of one batch element.

v2: fp8 rework of the bf16 baseline.
 - All projections (Q/K/V/O) run as fp8e4m3 DoubleRow matmuls: contraction of
   2x128 partitions per instruction at the same issue rate as bf16, i.e. 2x
   the throughput.
 - QK^T stays bf16 but alternates the 64-partition stationary base between
   head 0 (partitions 0:64) and head 1 (64:128) of each pair, so the PE
   loads one head's keys while streaming the other's queries (~2x issue rate).
 - Attention weights A are stored in fp8e4m3 with a global 2^-2 scale
   (K_OCT: raw scores reach ~75, which would overflow e4m3's 448 max and
   NaN byte 127; the scale moves both thresholds to ~80 and softmax
   normalization cancels it exactly). The exp stream - the biggest
   elementwise cost - is split between ScalarE (exact exp, fp8 output) and
   DVE (Schraudolph: affine in log2 domain + saturating round to uint8 IS the
   fp8e4m3 bit pattern). A@V contracts A against V (e4m3) in DoubleRow
   mode with a ones-column folding the softmax denominator into the matmul.
 - Denominators collect into [8,S] tiles; batched reciprocal + per-pair
   broadcast matmul (sel8) normalizes O^T, written fp8 for the O projection.
 - Residual + LayerNorm in fp32; the (x-mu)*rstd affine runs on ScalarE with
   per-partition scale/bias APs.
Zero biases / identity gamma,beta (checked on host) skip their instructions.
"""

import os
import sys

sys.path.insert(0, "/opt/trn_rl_repo")

import numpy as np

B, S, D, H = 4, 2048, 1024, 16
HD = D // H  # 64
SQ = S // 2  # queries per core
NCORES = 8
EPS = 1e-12

SHIFT = 2.5
K_OCT = 2  # global 2^-K scale on A: keeps e4m3 under NaN/448 for scores ~75
LOG2E = 1.4426950408889634
LN2 = 0.6931471805599453
NSHIFT = -(SHIFT + K_OCT * LN2)
A_DVE = LOG2E * 0.125 * 8               # schraudolph slope per raw score (e4m3)
B_DVE = 56.0 - 0.45 - 8 * SHIFT * LOG2E - 8 * K_OCT  # fitted offset (c=-0.45)

_CACHE = {}


def _install_ntff_hook():
    """Register the axon NTFF profile hook that bass_utils looks up via
    antenv.axon_hooks (absent from the image's antenv stub)."""
    import contextlib
    import ctypes
    import types

    so_path = "/opt/axon/libaxon_pjrt.so"
    if "antenv.axon_hooks" in sys.modules:
        return
    try:
        lib = ctypes.CDLL(so_path)
    except OSError:
        return
    if not hasattr(lib, "axon_start_nrt_profile"):
        return
    lib.axon_start_nrt_profile.argtypes = [ctypes.POINTER(ctypes.c_int64), ctypes.c_size_t]
    lib.axon_start_nrt_profile.restype = ctypes.c_int64
    lib.axon_stop_nrt_profile.argtypes = [ctypes.c_char_p]
    lib.axon_stop_nrt_profile.restype = ctypes.c_int64

    @contextlib.contextmanager
    def _hook(output_dir, device_ids):
        import jax

        jax.devices()
        if device_ids:
            ids = (ctypes.c_int64 * len(device_ids))(*device_ids)
            rc = lib.axon_start_nrt_profile(ids, len(device_ids))
        else:
            rc = lib.axon_start_nrt_profile(None, 0)
        if rc != 0:
            raise RuntimeError(f"axon_start_nrt_profile rc={rc}")
        try:
            yield
        finally:
            n = lib.axon_stop_nrt_profile(str(output_dir).encode())
            if n < 0:
                raise RuntimeError(f"axon_stop_nrt_profile rc={n}")

    m = types.ModuleType("antenv.axon_hooks")
    m.get_axon_ntff_profile_hook = lambda: _hook
    m.set_axon_ntff_profile_hook = lambda h: None
    sys.modules["antenv.axon_hooks"] = m


def _build(flags):
    use_bq, use_bk, use_bv, use_bo, use_gamma, use_beta = flags

    import concourse.bass as bass
    import concourse.tile as tile
    from concourse import bacc, mybir

    f32 = mybir.dt.float32
    bf16 = mybir.dt.bfloat16
    fp8 = mybir.dt.float8e4
    fp8e5 = mybir.dt.float8e5
    f32r = mybir.dt.float32r
    u8 = mybir.dt.uint8
    ADD = mybir.AluOpType.add
    MULT = mybir.AluOpType.mult
    SUB = mybir.AluOpType.subtract
    Exp = mybir.ActivationFunctionType.Exp
    Sqrt = mybir.ActivationFunctionType.Sqrt
    Copy = mybir.ActivationFunctionType.Copy
    Ident = mybir.ActivationFunctionType.Identity
    DR = mybir.MatmulPerfMode.DoubleRow

    nc = bacc.Bacc("TRN2")

    xt_d = nc.dram_tensor("xt", [D, S], fp8, kind="ExternalInput")
    xq_d = nc.dram_tensor("xq", [SQ, D], bf16, kind="ExternalInput")
    wq_d = nc.dram_tensor("wqt", [D, D], fp8, kind="ExternalInput")
    wk_d = nc.dram_tensor("wkt", [D, D], fp8, kind="ExternalInput")
    wv_d = nc.dram_tensor("wvt", [D, D], fp8, kind="ExternalInput")
    wo_d = nc.dram_tensor("wot", [D, D], fp8, kind="ExternalInput")
    bq_d = nc.dram_tensor("bqt", [128, 8], f32, kind="ExternalInput")
    bk_d = nc.dram_tensor("bkt", [128, 8], f32, kind="ExternalInput")
    bv_d = nc.dram_tensor("bv", [D], f32, kind="ExternalInput")
    bo_d = nc.dram_tensor("bo", [D], f32, kind="ExternalInput")
    gamma_d = nc.dram_tensor("gamma", [D], f32, kind="ExternalInput")
    beta_d = nc.dram_tensor("beta", [D], f32, kind="ExternalInput")
    sel8_d = nc.dram_tensor("sel8", [8, 4, 128], bf16, kind="ExternalInput")
    ident_d = nc.dram_tensor("ident", [128, 128], bf16, kind="ExternalInput")
    out_d = nc.dram_tensor("out", [SQ, D], f32, kind="ExternalOutput")

    def bcast_ap(handle):
        ap = handle[:]
        return bass.AP(tensor=ap.tensor, offset=ap.offset, ap=[[0, 128], ap.ap[0]])

    # which (hh, kc) score tiles go to ScalarE (exact exp) vs DVE (schraudolph)
    scalar_set = {i for i in range(32) if (i * 17) % 32 < 16}

    with tile.TileContext(nc) as tc:
        with (
            tc.tile_pool(name="const", bufs=1) as constp,
            tc.tile_pool(name="v", bufs=1) as vp,
            tc.tile_pool(name="ot", bufs=1) as otp,
            tc.tile_pool(name="xt", bufs=1) as xtp,
            tc.tile_pool(name="wo", bufs=1) as wop,
            tc.tile_pool(name="rc", bufs=1) as rcp,
        ):
            # --- constants ---
            bq_c = constp.tile([128, 8], f32, tag="bq")
            bk_c = constp.tile([128, 8], f32, tag="bk")
            eps_c = constp.tile([128, 1], f32, tag="eps")
            nshift_c = constp.tile([128, 1], f32, tag="nshift")
            scratch_c = constp.tile([128, 1], f32, tag="scratch")
            sel8_c = constp.tile([8, 4, 128], bf16, tag="sel8")
            ident = constp.tile([128, 128], bf16, tag="ident")
            bv_c = constp.tile([128, D], f32, tag="bv") if use_bv else None
            gamma_c = constp.tile([128, D], f32, tag="gamma") if use_gamma else None
            beta_c = constp.tile([128, D], f32, tag="beta") if use_beta else None
            bo_c = constp.tile([128, D], f32, tag="bo") if use_bo else None
            if use_bq:
                nc.sync.dma_start(out=bq_c[:], in_=bq_d[:])
            if use_bk:
                nc.sync.dma_start(out=bk_c[:], in_=bk_d[:])
            if use_bv:
                nc.gpsimd.dma_start(out=bv_c[:], in_=bcast_ap(bv_d))
            if use_bo:
                nc.gpsimd.dma_start(out=bo_c[:], in_=bcast_ap(bo_d))
            if use_gamma:
                nc.gpsimd.dma_start(out=gamma_c[:], in_=bcast_ap(gamma_d))
            if use_beta:
                nc.gpsimd.dma_start(out=beta_c[:], in_=bcast_ap(beta_d))
            nc.sync.dma_start(out=sel8_c[:], in_=sel8_d[:])
            nc.gpsimd.dma_start(out=ident[:], in_=ident_d[:])
            nc.vector.memset(eps_c[:], EPS)
            nc.vector.memset(nshift_c[:], NSHIFT)

            # --- persistent activations ---
            v8 = vp.tile([128, 16, H, HD + 1], fp8, tag="v")   # V + ones col (den)
            otb = otp.tile([128, 8, SQ], bf16, tag="otb")      # O^T unnormalized
            ot8 = otp.tile([128, 8, SQ], fp8, tag="ot8")       # O^T normalized
            den_a = otp.tile([8, SQ], f32, tag="den_a")        # heads 0-7
            den_b = otp.tile([4, SQ], f32, tag="den_b")        # heads 8-11
            den_c = otp.tile([4, SQ], f32, tag="den_c")        # heads 12-15
            xt = xtp.tile([128, 8, S], fp8, tag="xt")
            wo_r = wop.tile([128, 8, D], fp8, tag="wor")
            wv_r = wop.tile([128, 8, D], fp8, tag="wvr")
            xq_s = wop.tile([128, 8, D], bf16, tag="xq")

            nc.vector.memset(v8[:, :, :, HD : HD + 1], 1.0)
            # Single-post bulk loads: DMA posts cost ~700ns of engine time
            # each, so one 3D-AP transfer per tensor. pair-0 Q/K weights go
            # first on their queues (they gate the first matmuls).
            dq = [nc.sync, nc.scalar, nc.gpsimd]
            wq_0 = wop.tile([128, 8, 128], fp8, tag="wq0")
            wk_0 = wop.tile([128, 8, 128], fp8, tag="wk0")
            nc.sync.dma_start(
                out=wq_0[:], in_=wq_d[:, 0:128].rearrange("(k p) c -> p k c", p=128)
            )
            nc.scalar.dma_start(
                out=wk_0[:], in_=wk_d[:, 0:128].rearrange("(k p) c -> p k c", p=128)
            )
            # xt gates the first matmuls: keep sync/scalar queues clear for it;
            # wv (needed ~kc0 of pair 0) and wo (phase 3) ride on gpsimd.
            # xt rows spread over all three queues; wv/wo trail on gpsimd
            # (v_chain starts at kc 2, wo only needed in phase 3).
            for r in range(8):
                dq[r % 3].dma_start(
                    out=xt[:, r, :], in_=xt_d[r * 128 : (r + 1) * 128, :]
                )
            for k in range(4):
                nc.gpsimd.dma_start(
                    out=wv_r[:, 2 * k : 2 * k + 2, :],
                    in_=wv_d[2 * k * 128 : (2 * k + 2) * 128, :].rearrange(
                        "(k p) c -> p k c", p=128
                    ),
                )
            for k in range(4):
                nc.gpsimd.dma_start(
                    out=wo_r[:, 2 * k : 2 * k + 2, :],
                    in_=wo_d[2 * k * 128 : (2 * k + 2) * 128, :].rearrange(
                        "(k p) c -> p k c", p=128
                    ),
                )

            with (
                tc.tile_pool(name="qkw", bufs=2) as qkwp,
                tc.tile_pool(name="qts", bufs=2) as qtsp,
                tc.tile_pool(name="kts", bufs=2) as ktsp,
                tc.tile_pool(name="st", bufs=16) as stp,
                tc.tile_pool(name="stage", bufs=4) as stagep,
                tc.tile_pool(name="stgd", bufs=2) as stgdp,
                tc.tile_pool(name="ps1", bufs=1, space="PSUM") as ps1,
                tc.tile_pool(name="sp", bufs=3, space="PSUM") as spp,
                tc.tile_pool(name="av", bufs=1, space="PSUM") as avp,
            ):
                # ---------- piecewise emission helpers ----------
                def v_chain(tc_i, dg):
                    pool, tg = (ps1, "ps") if (2 * tc_i + dg) % 2 == 0 else (avp, "av")
                    psv = pool.tile([128, 512], f32, tag=tg, name="psv")
                    for k in range(4):
                        nc.tensor.matmul(
                            out=psv[:],
                            lhsT=xt[:, 2 * k : 2 * k + 2, tc_i * 128 : (tc_i + 1) * 128],
                            rhs=wv_r[:, 2 * k : 2 * k + 2, dg * 512 : (dg + 1) * 512],
                            start=(k == 0),
                            stop=(k == 3),
                            perf_mode=DR,
                        )
                    dst = v8[:, tc_i, dg * 8 : (dg + 1) * 8, 0:HD]
                    if use_bv:
                        nc.vector.tensor_tensor(
                            out=dst,
                            in0=psv[:].rearrange("p (h d) -> p h d", d=HD),
                            in1=bv_c[:, dg * 512 : (dg + 1) * 512].rearrange(
                                "p (h d) -> p h d", d=HD
                            ),
                            op=ADD,
                        )
                    else:
                        nc.vector.tensor_copy(
                            out=dst, in_=psv[:].rearrange("p (h d) -> p h d", d=HD)
                        )

                pair_qt = {}

                def proj_piece(m, j):
                    """j=0: wq DMA + Q chain tg0; j=1: Q tg1; j=2: wk DMA + K tg0;
                    j=3..5: K tg1..3."""
                    st = pair_qt.setdefault(m, {})
                    if j == 0:
                        if m == 0:
                            st["wq"] = wq_0
                        else:
                            wq_m = qkwp.tile([128, 8, 128], fp8, tag="qkw", name="wq_m")
                            nc.sync.dma_start(
                                out=wq_m[:],
                                in_=wq_d[:, m * 128 : (m + 1) * 128].rearrange(
                                    "(k p) c -> p k c", p=128
                                ),
                            )
                            st["wq"] = wq_m
                        st["qt"] = qtsp.tile([128, SQ], bf16, tag="qts", name="qt_m")
                    if j == 2:
                        if m == 0:
                            st["wk"] = wk_0
                        else:
                            wk_m = qkwp.tile([128, 8, 128], fp8, tag="qkw", name="wk_m")
                            nc.sync.dma_start(
                                out=wk_m[:],
                                in_=wk_d[:, m * 128 : (m + 1) * 128].rearrange(
                                    "(k p) c -> p k c", p=128
                                ),
                            )
                            st["wk"] = wk_m
                        st["kt"] = ktsp.tile([128, S], bf16, tag="kts", name="kt_m")
                    if j < 2:
                        w, dstt, tg, bias_c, use_b = st["wq"], st["qt"], j, bq_c, use_bq
                    else:
                        w, dstt, tg, bias_c, use_b = st["wk"], st["kt"], j - 2, bk_c, use_bk
                    ps = ps1.tile([128, 512], f32, tag="ps", name="psqk")
                    for k in range(4):
                        nc.tensor.matmul(
                            out=ps[:],
                            lhsT=w[:, 2 * k : 2 * k + 2, :],
                            rhs=xt[:, 2 * k : 2 * k + 2, tg * 512 : (tg + 1) * 512],
                            start=(k == 0),
                            stop=(k == 3),
                            perf_mode=DR,
                        )
                    if use_b:
                        nc.scalar.activation(
                            out=dstt[:, tg * 512 : (tg + 1) * 512],
                            in_=ps[:],
                            func=Ident,
                            bias=bias_c[:, m : m + 1],
                        )
                    else:
                        nc.scalar.copy(
                            out=dstt[:, tg * 512 : (tg + 1) * 512], in_=ps[:]
                        )

                def qk_exp_kc(m, kc, qt_m, kt_m, st_pair):
                    sps = [
                        spp.tile([128, 1024], f32, tag="sp", name="sp") for _ in range(2)
                    ]
                    for hh in range(2):
                        p0 = hh * 64
                        for qh in range(2):
                            nc.tensor.matmul(
                                out=sps[hh][:, qh * 512 : (qh + 1) * 512],
                                lhsT=kt_m[p0 : p0 + 64, kc * 128 : (kc + 1) * 128],
                                rhs=qt_m[p0 : p0 + 64, qh * 512 : (qh + 1) * 512],
                                start=True,
                                stop=True,
                            )
                        dst = st_pair[hh][kc // 4][:, kc % 4, :]
                        if (2 * kc + hh) in scalar_set:
                            nc.scalar.activation(
                                out=dst,
                                in_=sps[hh][:],
                                func=Exp,
                                scale=0.125,
                                bias=nshift_c[:, 0:1],
                            )
                        else:
                            nc.vector.tensor_scalar(
                                out=dst.bitcast(u8),
                                in0=sps[hh][:],
                                scalar1=float(A_DVE),
                                scalar2=float(B_DVE),
                                op0=MULT,
                                op1=ADD,
                            )

                av_stg = {}
                av_live = {}

                def av_half(m, half_i, st_pair, tail=False):
                    """half_i = 0..7: piece (hh,qh) split into two 4-accum halves."""
                    piece, half = half_i // 2, half_i % 2
                    den_t = den_a if m < 4 else (den_b if m < 6 else den_c)
                    den_r = 2 * (m % 4) if m < 4 else (2 * (m - 4) if m < 6 else 2 * (m - 6))
                    hh, qh = piece // 2, piece % 2
                    h = 2 * m + hh
                    st_tiles = st_pair[hh]
                    if qh == 0 and half == 0:
                        av_stg[(m, hh)] = stagep.tile(
                            [65, 2, 512], bf16, tag="stg", name="stg"
                        )
                        av_stg[(m, hh, "d")] = stgdp.tile(
                            [65, 2, 512], f32, tag="stgd", name="stgd"
                        )
                    stg = av_stg[(m, hh)]
                    stgd = av_stg[(m, hh, "d")]
                    if half == 0:
                        if tail:
                            # QK is done; reuse a freed score bank so tail
                            # pieces pipeline instead of serializing on avp.
                            av_live[m] = spp.tile([128, 1024], f32, tag="sp", name="sp")
                        else:
                            av_live[m] = avp.tile([128, 512], f32, tag="av", name="av")
                    av = av_live[m]
                    for c in range(4 * half, 4 * half + 4):
                        u, jj = c // 2, c % 2
                        nc.tensor.matmul(
                            out=av[0:65, 0:512],
                            lhsT=v8[:, 4 * u + 2 * jj : 4 * u + 2 * jj + 2, h, :],
                            rhs=st_tiles[u][:, 2 * jj : 2 * jj + 2, qh * 512 : (qh + 1) * 512],
                            start=(c == 0),
                            stop=(c == 7),
                            perf_mode=DR,
                            skip_group_check=True,
                        )
                    if half == 0:
                        return
                    if hh == 0:
                        nc.vector.tensor_copy(
                            out=otb[0:64, m, qh * 512 : (qh + 1) * 512],
                            in_=av[0:64, 0:512],
                        )
                    else:
                        nc.scalar.copy(out=stg[0:64, qh, :], in_=av[0:64, 0:512])
                    nc.scalar.copy(out=stgd[64:65, qh, :], in_=av[64:65, 0:512])
                    if qh == 1:
                        nc.sync.dma_start(
                            out=den_t[den_r + hh : den_r + hh + 1, :],
                            in_=stgd[64:65, :, :],
                        )
                        if hh == 1:
                            nc.sync.dma_start(
                                out=otb[64:128, m, :], in_=stg[0:64, :, :]
                            )

                NORM_PAIRS = ((0, 1, 2, 3), (4, 5), (6, 7))
                norm_rc = {}

                def norm_prep(b_i, r0=0, r1=None):
                    # DVE-side reciprocal for a den batch; emitted at pair end
                    den_t = (den_a, den_b, den_c)[b_i]
                    if r1 is None:
                        r1 = (8, 4, 4)[b_i]
                    rc_f = rcp.tile([8, SQ], f32, tag="rcf", name="rc_f")
                    rc_b = rcp.tile([8, SQ], bf16, tag="rcb", name="rc_b")
                    nc.vector.reciprocal_approx_fast(
                        out=rc_f[r0:r1, :], in_=den_t[r0:r1, :]
                    )
                    nc.vector.tensor_copy(out=rc_b[r0:r1, :], in_=rc_f[r0:r1, :])
                    norm_rc[b_i] = rc_b

                def norm_bc(b_i, i, pool=None, ptag="ps"):
                    # one broadcast matmul + normalize; spread into later kc slots
                    pairs = NORM_PAIRS[b_i]
                    m, qh = pairs[i // 2], i % 2
                    mm = m % 4 if b_i == 0 else m - (4 if b_i == 1 else 6)
                    rc_b = norm_rc[b_i]
                    bc = (pool or ps1).tile([128, 512], f32, tag=ptag, name="bc")
                    nc.tensor.matmul(
                        out=bc[:],
                        lhsT=sel8_c[:, mm, :],
                        rhs=rc_b[:, qh * 512 : (qh + 1) * 512],
                        start=True,
                        stop=True,
                    )
                    nc.vector.tensor_tensor(
                        out=ot8[:, m, qh * 512 : (qh + 1) * 512],
                        in0=otb[:, m, qh * 512 : (qh + 1) * 512],
                        in1=bc[:],
                        op=MULT,
                    )

                # ---------- interleaved pipeline ----------
                pair_st = {}
                vq = [(tc_i, dg) for tc_i in range(16) for dg in range(2)]
                # only Q (j=0,1) and K tg0 (j=2) up front; K tg1..3 fold into
                # kc 1/3/5 of pair 0 so QK can start on partial kt.
                for jj in range(3):
                    proj_piece(0, jj)
                for m in range(8):
                    qt_m = pair_qt[m]["qt"]
                    kt_m = pair_qt[m]["kt"]
                    st_pair = [
                        [stp.tile([128, 4, SQ], fp8, tag="st", name="st") for _ in range(4)]
                        for _ in range(2)
                    ]
                    pair_st[m] = st_pair
                    for kc in range(16):
                        qk_exp_kc(m, kc, qt_m, kt_m, st_pair)
                        if m == 0:
                            if kc in (1, 3, 5):
                                proj_piece(0, 3 + kc // 2)
                            # fold the V projection into pair 0's loop
                            # (start at kc 2 so wv's DMA has landed)
                            nv = 0 if kc < 2 else (3 if kc < 6 else 2)
                            for _ in range(nv):
                                if vq:
                                    v_chain(*vq.pop(0))
                        if m >= 1 and kc % 2 == 1:
                            av_half(m - 1, kc // 2, pair_st[m - 1])
                        if m == 5 and kc % 2 == 1:
                            norm_bc(0, kc // 2)
                        if m == 7 and kc in (3, 5, 7, 9):
                            norm_bc(1, (kc - 3) // 2)
                        if m < 7 and kc % 2 == 0 and kc < 12:
                            proj_piece(m + 1, kc // 2)
                        if m == 5 and kc == 0:
                            # prefetch X rows for the phase-3 residual
                            nc.gpsimd.dma_start(
                                out=xq_s[:],
                                in_=xq_d[:].rearrange("(t p) c -> p t c", p=128),
                            )
                    if m >= 2:
                        del pair_st[m - 2]
                    if m == 4:
                        norm_prep(0)
                    if m == 6:
                        norm_prep(1)
                norm_prep(2, 0, 2)  # pair 6 (den ready); pair 7 comes in phase 3
                for hi in range(8):
                    av_half(7, hi, pair_st[7], tail=True)
                    if hi in (3, 5):
                        norm_bc(2, (hi - 3) // 2)
                # pull the sqrt activation table in before phase 3 needs it
                nc.scalar.activation(
                    out=scratch_c[:], in_=eps_c[:], func=Sqrt, bias=eps_c[:, 0:1]
                )

            # ========== phase 3: O projection + residual + LN ==========
            with (
                tc.tile_pool(name="yo", bufs=4) as yop,
                tc.tile_pool(name="stats", bufs=4) as statp,
                tc.tile_pool(name="ps3", bufs=4, space="PSUM") as ps3,
            ):
                if use_bo:
                    for t in range(8):
                        nc.gpsimd.tensor_tensor(
                            out=xq_s[:, t, :], in0=xq_s[:, t, :], in1=bo_c[:], op=ADD
                        )
                def oproj_mm(ps, t, g, k):
                    nc.tensor.matmul(
                        out=ps[:, g * 512 : (g + 1) * 512],
                        lhsT=ot8[:, 2 * k : 2 * k + 2, t * 128 : (t + 1) * 128],
                        rhs=wo_r[:, 2 * k : 2 * k + 2, g * 512 : (g + 1) * 512],
                        start=(k == 0),
                        stop=False,
                        perf_mode=DR,
                        skip_group_check=True,
                    )

                # allocate bc7's PSUM slot ahead of t0 so it recycles first
                # and the t-loop keeps full 4-deep ps3 rotation
                bc7 = ps3.tile([128, D], f32, tag="ps3", name="bc7")
                for t in range(8):
                    ps = ps3.tile([128, D], f32, tag="ps3", name="ps3")
                    stats = statp.tile([128, 2, 6], f32, tag="stats")
                    mv = statp.tile([128, 2], f32, tag="mv")
                    mr = statp.tile([128, 1], f32, tag="mr")
                    if t == 0:
                        # pairs 0-5 first: pair-7 normalization in flight
                        for g in range(2):
                            for k in range(3):
                                oproj_mm(ps, t, g, k)
                        # (redo rows 0:2 too: DVE APs must start at partition 0)
                        norm_prep(2, 0, 4)
                        for qh in range(2):
                            nc.tensor.matmul(
                                out=bc7[:, qh * 512 : (qh + 1) * 512],
                                lhsT=sel8_c[:, 1, :],
                                rhs=norm_rc[2][:, qh * 512 : (qh + 1) * 512],
                                start=True,
                                stop=True,
                            )
                            nc.vector.tensor_tensor(
                                out=ot8[:, 7, qh * 512 : (qh + 1) * 512],
                                in0=otb[:, 7, qh * 512 : (qh + 1) * 512],
                                in1=bc7[:, qh * 512 : (qh + 1) * 512],
                                op=MULT,
                            )
                        for g in range(2):
                            oproj_mm(ps, t, g, 3)
                            nc.tensor.matmul(
                                out=ps[:, g * 512 : (g + 1) * 512],
                                lhsT=ident[:],
                                rhs=xq_s[:, t, g * 512 : (g + 1) * 512],
                                start=False,
                                stop=True,
                                skip_group_check=True,
                            )
                            nc.vector.bn_stats(
                                out=stats[:, g, :],
                                in_=ps[:, g * 512 : (g + 1) * 512],
                            )
                    else:
                        # g-major: half-g stats overlap the other half's matmuls
                        for g in range(2):
                            for k in range(4):
                                oproj_mm(ps, t, g, k)
                            # residual: accumulate X via bf16 identity matmul
                            nc.tensor.matmul(
                                out=ps[:, g * 512 : (g + 1) * 512],
                                lhsT=ident[:],
                                rhs=xq_s[:, t, g * 512 : (g + 1) * 512],
                                start=False,
                                stop=True,
                                skip_group_check=True,
                            )
                            nc.vector.bn_stats(
                                out=stats[:, g, :],
                                in_=ps[:, g * 512 : (g + 1) * 512],
                            )
                    nc.vector.bn_aggr(out=mv[:], in_=stats[:])
                    nc.scalar.activation(
                        out=mv[:, 1:2], in_=mv[:, 1:2], func=Sqrt, bias=eps_c[:, 0:1]
                    )
                    nc.vector.reciprocal(out=mv[:, 1:2], in_=mv[:, 1:2])
                    nc.vector.tensor_scalar(
                        out=mr[:],
                        in0=mv[:, 0:1],
                        scalar1=mv[:, 1:2],
                        scalar2=-1.0,
                        op0=MULT,
                        op1=MULT,
                    )
                    yo = yop.tile([128, D], f32, tag="yo")
                    nc.scalar.activation(
                        out=yo[:], in_=ps[:], func=Ident,
                        scale=mv[:, 1:2], bias=mr[:, 0:1],
                    )
                    if use_gamma:
                        nc.vector.tensor_tensor(out=yo[:], in0=yo[:], in1=gamma_c[:], op=MULT)
                    if use_beta:
                        nc.gpsimd.tensor_tensor(out=yo[:], in0=yo[:], in1=beta_c[:], op=ADD)
                    dq[t % 3].dma_start(
                        out=out_d[t * 128 : (t + 1) * 128, :], in_=yo[:]
                    )

    nc.compile()
    return nc


def _get_nc(flags):
    key = ("nc", flags)
    if key not in _CACHE:
        _CACHE[key] = _build(flags)
    return _CACHE[key]


def kernel(X, Wq, bq, Wk, bk, Wv, bv, Wo, bo, gamma, beta):
    if os.environ.get("BASS_TRACE"):
        _install_ntff_hook()
    import ml_dtypes

    from concourse.bass_utils import run_bass_kernel_spmd

    f8 = ml_dtypes.float8_e4m3
    bfdt = ml_dtypes.bfloat16
    f32 = np.float32
    X = np.ascontiguousarray(np.asarray(X, dtype=f32))
    bq_ = np.asarray(bq, f32)
    bk_ = np.asarray(bk, f32)
    bv_ = np.ascontiguousarray(np.asarray(bv, f32))
    bo_ = np.ascontiguousarray(np.asarray(bo, f32))
    gamma_ = np.ascontiguousarray(np.asarray(gamma, f32))
    beta_ = np.ascontiguousarray(np.asarray(beta, f32))
    flags = (
        bool(np.any(bq_)), bool(np.any(bk_)), bool(np.any(bv_)), bool(np.any(bo_)),
        bool(np.any(gamma_ != 1.0)), bool(np.any(beta_)),
    )

    wqt = np.ascontiguousarray(np.asarray(Wq, f32).T.astype(f8))
    wkt = np.ascontiguousarray(np.asarray(Wk, f32).T.astype(f8))
    wvt = np.ascontiguousarray(np.asarray(Wv, f32).T.astype(f8))
    wot = np.ascontiguousarray(np.asarray(Wo, f32).T.astype(f8))
    bqt = np.ascontiguousarray(bq_.reshape(8, 128).T)
    bkt = np.ascontiguousarray(bk_.reshape(8, 128).T)
    sel8 = np.zeros((8, 4, 128), f32)
    for mm in range(4):
        for p in range(128):
            sel8[2 * mm + p // 64, mm, p] = 1.0
    sel8 = sel8.astype(bfdt)
    ident = np.eye(128, dtype=f32).astype(bfdt)

    in_maps = []
    for c in range(NCORES):
        b, half = c // 2, c % 2
        Xb = X[b]
        q_rows = Xb[half * SQ : (half + 1) * SQ]
        o_rows = Xb[(1 - half) * SQ : (2 - half) * SQ]
        # queries-first token order (key order is permutation-invariant)
        xt = np.ascontiguousarray(np.concatenate([q_rows, o_rows], axis=0).T.astype(f8))
        in_maps.append(
            {
                "xt": xt,
                "xq": np.ascontiguousarray(q_rows.astype(bfdt)),
                "wqt": wqt,
                "wkt": wkt,
                "wvt": wvt,
                "wot": wot,
                "bqt": bqt,
                "bkt": bkt,
                "bv": bv_,
                "bo": bo_,
                "gamma": gamma_,
                "beta": beta_,
                "sel8": sel8,
                "ident": ident,
            }
        )

    nc = _get_nc(flags)
    res = run_bass_kernel_spmd(nc, in_maps, core_ids=list(range(NCORES)))
    if res.exec_time_ns is not None:
        print(f"HW exec time: {res.exec_time_ns} ns")

    out = np.empty((B, S, D), np.float32)
    for c in range(NCORES):
        b, half = c // 2, c % 2
        out[b, half * SQ : (half + 1) * SQ] = res.results[c]["out"]
    return out

